# revision 37
# baseline (speedup 1.0000x reference)
"""Trainium2 Bass kernel for CompiledNCA — row-Toeplitz formulation.

Data parallel over batch (128 imgs -> 8 cores x 16; per core 2 groups of 8).
SBUF layout: partitions = (slot j, ci), j in [0,8); slot j of segment s holds
padded image row 6s + PI[j] with PI = [0,2,1,3,6,4,7,5]; free = (img, seg, x)
with x the 132-strided padded row (cols 0..129). Segments overlap 2 rows
(rows 6s, 6s+1 are stored twice); PI puts the 4 seam-row slots (rows 0,1,6,7)
at even slots so seam-replication copies start at 32-aligned partitions.

Each conv step + seg = 3 PSUM-accumulated full-array matmuls (one per
kernel-x tap dx, free-dim shifted); lhsT is a banded block-Toeplitz
[128=(jr,ci), 128=(jo,co)] holding W[co,ci,ky,dx], ky = PI[jr]-PI[jo]+1;
output slots that are not computable from the 8-row window (PI[jo] in {0,7})
have zero columns. One 512-col stream computes 6 output rows x 16 ch (2.18x
fewer PE cycles than a block-diagonal-per-image formulation). ReLU evac is a
full-partition psum->SBUF copy (no partition offset); seam rows are then
replicated by two batched fp16 SBUF->SBUF copies per image (which also
overwrite the zero columns with the true rows). Segment 21 (rows 127..128)
runs as its own N=128 matmul with a column-masked lhsT so halo slots stay 0.
Stem: dx-im2col'd x on partitions (yi,dx), 24-partition contract. Pool/fc:
ACT accum_out (DVE tensor_reduce for the DVE-evac'd groups; DVE accum_out is
low-precision) -> tensor_reduce -> one matmul + bias.

Schedule (PE was 91%-busy and DVE 88% in the v1 timeline sim; now PE ~98%):
- Evacs split ACT (seg-groups k<4) / DVE (k>=4 + both seam copies) so DVE is
  no longer the co-bottleneck. gpsimd cannot read PSUM, so it cannot help.
- Last step's h write is dead (only pooling reads it) and is recycled into
  the just-read src regions, freeing the other hbuf so group 1's stem
  (evac-bound: 1 matmul per seg-group vs 3) interleaves across group 0's
  t=14..15; group 0's stem staggers 2 images ahead of its own t=0.
- fc for group 0 is emitted during group 1's t=0; group 1's fc is split
  (imgs 0-5 early, 6-7 last) so only 2 images gate the final chain.
- In multi-rep NEFFs (timing), the next rep's group-0 stem interleaves into
  this rep's group-1 t=14..15, pipelining the rep boundary the same way.
- Startup: input DMA in 2-image slices ordered stem-first; hbuf memset
  narrowed to the pad columns; both group accumulators pre-allocated.
"""

import numpy as np

B, HC, OC, T = 128, 16, 10, 16
H = W = 128
NCORES = 8
IMGS = 8
GROUPS = 2
NSEG = 22                     # segments of 6 output rows
XST = 132                     # x stride per (img,seg); cols 0..129 meaningful
SEGF = NSEG * XST             # 2904 free cols per img
MCOL = 128
PI = [0, 2, 1, 3, 6, 4, 7, 5]            # slot j -> row offset within segment
INV = [PI.index(r) for r in range(8)]    # row offset -> slot
SEG_T = [(0, 4), (4, 4), (8, 4), (12, 4), (16, 4), (20, 1), (21, 1)]

_cache = {}


def _view(hb, p0, np_, base, ns, x0, xn):
    """[np_, ns, xn] view of flat (seg,x) storage starting at free `base`."""
    v = hb[p0:p0 + np_, base:base + ns * XST]
    return v.rearrange("p (s x) -> p s x", x=XST)[:, :, x0:x0 + xn]


def _build(repeat=1):
    if repeat in _cache:
        return _cache[repeat]

    import concourse.bacc as bacc
    import concourse.mybir as mybir
    import concourse.tile as tile

    f16, f32 = mybir.dt.float16, mybir.dt.float32
    Relu = mybir.ActivationFunctionType.Relu
    Ident = mybir.ActivationFunctionType.Identity

    nc = bacc.Bacc("TRN2", target_bir_lowering=False, debug=False,
                   enable_asserts=False, num_devices=NCORES)

    d_w = nc.dram_tensor("w_steps", [128, T * 6 * 128], f16, kind="ExternalInput")
    d_stemw = nc.dram_tensor("stem_w", [24, 2 * 128], f16, kind="ExternalInput")
    d_stemb = nc.dram_tensor("stem_b", [128, 2], f32, kind="ExternalInput")
    d_fcw = nc.dram_tensor("fc_w", [128, 128], f32, kind="ExternalInput")
    d_fcb = nc.dram_tensor("fc_b", [OC, 1], f32, kind="ExternalInput")
    d_xe = [nc.dram_tensor(f"x_exp{g}", [24, IMGS * SEGF], f16,
                           kind="ExternalInput") for g in range(GROUPS)]
    d_out = nc.dram_tensor("out", [GROUPS * OC, IMGS], f32, kind="ExternalOutput")

    with tile.TileContext(nc) as tc:
        with tc.tile_pool(name="const", bufs=1) as cp, \
             tc.tile_pool(name="hbuf", bufs=1) as hp, \
             tc.tile_pool(name="small", bufs=2) as sp, \
             tc.tile_pool(name="psum", bufs=8, space="PSUM") as pp:

            stemw_sb = cp.tile([24, 2 * 128], f16, tag="sw")
            nc.sync.dma_start(stemw_sb[:], d_stemw[:])
            stemb_sb = cp.tile([128, 2], f32, tag="sb")
            nc.sync.dma_start(stemb_sb[:], d_stemb[:])
            # per-image DMA slices so the stem can start after the first one
            xe_sb = []
            for g in range(GROUPS):
                t_ = cp.tile([24, IMGS * SEGF], f16, tag=f"xe{g}", name=f"xe{g}")
                xe_sb.append(t_)
            for i in range(0, IMGS, 2):
                nc.sync.dma_start(xe_sb[0][:, i * SEGF:(i + 2) * SEGF],
                                  d_xe[0][:, i * SEGF:(i + 2) * SEGF])
            w_tiles = []
            for t in range(T):
                t_ = cp.tile([128, 6 * 128], f16, tag=f"w{t}", name=f"w{t}")
                nc.sync.dma_start(t_[:], d_w[:, t * 6 * 128:(t + 1) * 6 * 128])
                w_tiles.append(t_)
            for i in range(0, IMGS, 2):
                nc.sync.dma_start(xe_sb[1][:, i * SEGF:(i + 2) * SEGF],
                                  d_xe[1][:, i * SEGF:(i + 2) * SEGF])
            fcw_sb = cp.tile([128, 128], f32, tag="fw")
            nc.sync.dma_start(fcw_sb[:], d_fcw[:])
            fcb_sb = cp.tile([OC, 1], f32, tag="fb")
            nc.sync.dma_start(fcb_sb[:], d_fcb[:])

            hbufs = [hp.tile([128, IMGS * SEGF], f16, tag=f"h{i}", name=f"h{i}")
                     for i in range(2)]
            for hb in hbufs:
                # only pad cols 0 and 129 of each (img, seg) row need zeros;
                # everything else is written before it is read
                pads = hb[:].rearrange("p (q x) -> p q x", x=XST)
                nc.vector.memset(pads[:, :, 0:130:129], 0.0)

            accs = [sp.tile([128, IMGS * 7], f32, tag="acc", name=f"acc{g}")
                    for g in range(GROUPS)]


            def step_img(g, t, i):
                src = xe_sb[g] if t < 0 else hbufs[t % 2]
                # at t==T-1 the h write is dead (only pooling reads it), so
                # recycle the just-read src regions: keeps hbuf[(t+1)%2] free
                # for the next group's stem to overlap t=14..15
                dst = hbufs[(t + 1) % 2] if t != T - 1 else hbufs[t % 2]
                acc = accs[g]
                if True:
                    for k, (s0, ns) in enumerate(SEG_T):
                        N = ns * 128
                        sl = 1 if s0 + ns == NSEG else 0  # seg-21 variant
                        ps = pp.tile([128, 512], f32, tag="ps")
                        if t < 0:
                            rhs = _view(src, 0, 24,
                                        i * SEGF + s0 * XST, ns, 0, 128)
                            nc.tensor.matmul(
                                ps[:, 0:N],
                                stemw_sb[:, sl * 128:sl * 128 + 128], rhs,
                                start=True, stop=True)
                        else:
                            for dx in range(3):
                                rhs = _view(src, 0, 128,
                                            i * SEGF + s0 * XST, ns, dx, 128)
                                wv = w_tiles[t][:, (sl * 3 + dx) * 128:
                                                (sl * 3 + dx + 1) * 128]
                                nc.tensor.matmul(ps[:, 0:N], wv, rhs,
                                                 start=(dx == 0),
                                                 stop=(dx == 2))
                        ps3 = ps[:, 0:N].rearrange("p (s x) -> p s x", x=128)
                        dmain = _view(dst, 0, 128,
                                      i * SEGF + s0 * XST, ns, 1, 128)
                        kwargs = {}
                        if t == T - 1:
                            kwargs = dict(
                                accum_out=acc[:, i * 7 + k:i * 7 + k + 1])
                        # evac split: ACT takes the first 4 seg-groups, DVE
                        # the rest (DVE also owns the seam copies); gpsimd
                        # cannot read PSUM so only ACT/DVE evacuate
                        if t < 0:
                            if k < 4:
                                nc.scalar.activation(
                                    dmain, ps3, Relu,
                                    bias=stemb_sb[:, sl:sl + 1], **kwargs)
                            else:
                                nc.vector.tensor_scalar(
                                    dmain, ps3, stemb_sb[:, sl:sl + 1], 0.0,
                                    mybir.AluOpType.add, mybir.AluOpType.max,
                                    **kwargs)
                        elif k < 4:
                            nc.scalar.activation(dmain, ps3, Relu, **kwargs)
                        elif t == T - 1:
                            # relu to the (dead) h slot, then DVE's
                            # fp32-internal tensor_reduce (accum_out on DVE
                            # is low-precision)
                            nc.vector.tensor_scalar_max(dmain, ps3, 0.0)
                            nc.vector.tensor_reduce(
                                acc[:, i * 7 + k:i * 7 + k + 1], dmain,
                                axis=mybir.AxisListType.XY,
                                op=mybir.AluOpType.add)
                        else:
                            nc.vector.tensor_scalar_max(dmain, ps3, 0.0)
                    if t == T - 1:
                        return  # h no longer read; pooling via accum_out
                    # seam replication (fp16 sbuf->sbuf, batched over segs);
                    # cols 0 and 129 are zero on both sides (memset pads), so
                    # copy only 1..128
                    # dup5: row 6s+6 (slot 4, s=0..20) -> slot 0 of seg s+1
                    nc.vector.tensor_copy(
                        _view(dst, 0, 16, i * SEGF + XST, 21, 1, 128),
                        _view(dst, 64, 16, i * SEGF, 21, 1, 128))
                    # dup0: row 6s+1 (slot 2, s=1..21) -> slot 6 of seg s-1
                    nc.vector.tensor_copy(
                        _view(dst, 96, 16, i * SEGF, 21, 1, 128),
                        _view(dst, 32, 16, i * SEGF + XST, 21, 1, 128))

            def fc_tail(g, lo=0, hi=IMGS, pooled=None, out_sb=None):
                n = hi - lo
                if pooled is None:
                    pooled = sp.tile([128, IMGS], f32, tag="pooled",
                                     name=f"pooled{g}")
                    out_sb = sp.tile([OC, IMGS], f32, tag="osb",
                                     name=f"osb{g}")
                nc.vector.tensor_reduce(
                    pooled[:, lo:hi],
                    accs[g][:, lo * 7:hi * 7].rearrange(
                        "p (i k) -> p i k", k=7),
                    axis=mybir.AxisListType.X, op=mybir.AluOpType.add)
                psfc = pp.tile([128, 512], f32, tag="ps", name=f"psfc{g}_{lo}")
                nc.tensor.matmul(psfc[:, 0:n], fcw_sb[:], pooled[:, lo:hi],
                                 start=True, stop=True)
                nc.scalar.activation(out_sb[:, lo:hi], psfc[0:OC, 0:n], Ident,
                                     bias=fcb_sb[:])
                nc.sync.dma_start(d_out[g * OC:(g + 1) * OC, lo:hi],
                                  out_sb[:, lo:hi])
                return pooled, out_sb

            for _rep in range(repeat):
                for g in range(GROUPS):
                    nc.vector.memset(accs[g][:], 0.0)
                if _rep == 0:
                    # first rep only: stem is evac-bound (1 matmul per group
                    # vs 3), so stagger t=0 two images behind the stem
                    step_img(0, -1, 0)
                    step_img(0, -1, 1)
                    for i in range(IMGS):
                        if i + 2 < IMGS:
                            step_img(0, -1, i + 2)
                        step_img(0, 0, i)
                else:
                    # stem already ran, interleaved into the previous rep's
                    # group-1 t=14..15
                    for i in range(IMGS):
                        step_img(0, 0, i)
                for t in range(1, T - 2):
                    for i in range(IMGS):
                        step_img(0, t, i)
                # group 1's stem (evac-bound) spreads over group 0's last two
                # steps; legal because t=15 writes recycle the src buffer
                for i in range(IMGS):
                    step_img(0, T - 2, i)
                    if i % 2 == 1:
                        step_img(1, -1, i // 2)
                for i in range(IMGS):
                    step_img(0, T - 1, i)
                    if i % 2 == 1:
                        step_img(1, -1, 4 + i // 2)
                for t in range(0, T - 2):
                    for i in range(IMGS):
                        step_img(1, t, i)
                    if t == 0:
                        fc_tail(0)  # overlaps group-1 compute
                # group-1 t=14..15 host the NEXT rep's group-0 stem (same
                # src-recycling argument as the group boundary above)
                nxt = _rep + 1 < repeat
                for i in range(IMGS):
                    step_img(1, T - 2, i)
                    if nxt and i % 2 == 1:
                        step_img(0, -1, i // 2)
                for i in range(IMGS):
                    step_img(1, T - 1, i)
                    if nxt and i % 2 == 1:
                        step_img(0, -1, 4 + i // 2)
                # fc for images 0..5 goes first (their accums are done while
                # imgs 6-7 compute); only images 6..7 gate the final chain
                p1, o1 = fc_tail(1, 0, 6)
                fc_tail(1, 6, IMGS, p1, o1)

    nc.compile()
    _cache[repeat] = nc
    return nc


def _prep_shared(stem_weight, stem_bias, weight_schedule, fc_weight, fc_bias):
    w = weight_schedule.astype(np.float32)          # [T, co, ci, 3, 3]
    # [(t, sl, dx), (jr,ci), (jo,co)]; sl=1 is the seg-21 column-masked variant
    lhs = np.zeros((T, 2, 3, 128, 128), np.float16)
    for sl in range(2):
        rows_ok = (1, 2) if sl else (1, 2, 3, 4, 5, 6)
        for dx in range(3):
            for jo in range(8):
                if PI[jo] not in rows_ok:
                    continue
                for jr in range(8):
                    ky = PI[jr] - PI[jo] + 1
                    if 0 <= ky <= 2:
                        lhs[:, sl, dx, jr * HC:(jr + 1) * HC,
                            jo * HC:(jo + 1) * HC] = \
                            np.transpose(w[:, :, :, ky, dx], (0, 2, 1))
    w_steps = np.ascontiguousarray(
        np.transpose(lhs, (3, 0, 1, 2, 4)).reshape(128, T * 6 * 128))

    sw = stem_weight.astype(np.float32)             # [HC, 1, 3, 3]
    stem_lhs = np.zeros((24, 2, 128), np.float16)
    stem_b = np.zeros((128, 2), np.float32)
    for sl in range(2):
        rows_ok = (1, 2) if sl else (1, 2, 3, 4, 5, 6)
        for jo in range(8):
            if PI[jo] not in rows_ok:
                continue
            stem_b[jo * HC:(jo + 1) * HC, sl] = stem_bias
            for yi in range(8):
                ky = yi - PI[jo] + 1
                if 0 <= ky <= 2:
                    for dx in range(3):
                        stem_lhs[yi * 3 + dx, sl, jo * HC:(jo + 1) * HC] = \
                            sw[:, 0, ky, dx]
    stem_lhs = np.ascontiguousarray(stem_lhs.reshape(24, 256))

    fcw = np.tile(fc_weight.astype(np.float32).T / float(H * W), (8, 1))
    fcw = np.concatenate([fcw, np.zeros((128, 128 - OC), np.float32)], axis=1)
    fc_b = fc_bias.astype(np.float32)[:, None].copy()
    return {"w_steps": w_steps, "stem_w": stem_lhs, "stem_b": stem_b,
            "fc_w": fcw, "fc_b": fc_b}


def _prep_xexp(x_imgs):
    """Stem input: [yi*3+dx, (img, seg, 132)] fp16; AP x-offset j reads
    xpad[img, 6s+yi, j+dx]."""
    xpad = np.zeros((IMGS, H + 2, XST), np.float32)
    xpad[:, 1:1 + H, 1:1 + W] = x_imgs[:, 0]
    out = np.zeros((24, IMGS * SEGF), np.float16)
    for i in range(IMGS):
        for yi in range(8):
            for dx in range(3):
                p = yi * 3 + dx
                for s in range(NSEG):
                    r = 6 * s + yi
                    if r >= H + 2:
                        continue
                    col = i * SEGF + s * XST
                    out[p, col:col + 130 - dx] = xpad[i, r, dx:130]
    return out


def kernel(x, stem_weight, stem_bias, weight_schedule, fc_weight, fc_bias):
    from concourse.bass_utils import run_bass_kernel_spmd

    x = np.asarray(x, dtype=np.float32)
    nc = _build()
    shared = _prep_shared(np.asarray(stem_weight, np.float32),
                          np.asarray(stem_bias, np.float32),
                          np.asarray(weight_schedule, np.float32),
                          np.asarray(fc_weight, np.float32),
                          np.asarray(fc_bias, np.float32))
    in_maps = []
    for c in range(NCORES):
        m = dict(shared)
        for g in range(GROUPS):
            lo = c * GROUPS * IMGS + g * IMGS
            m[f"x_exp{g}"] = _prep_xexp(x[lo:lo + IMGS])
        in_maps.append(m)

    res = run_bass_kernel_spmd(nc, in_maps, core_ids=list(range(NCORES)),
                               trace=False)
    outs = []
    for c in range(NCORES):
        o = res.results[c]["out"]          # [G*OC, IMGS]
        for g in range(GROUPS):
            outs.append(o[g * OC:(g + 1) * OC, :].T)
    return np.concatenate(outs, axis=0).astype(np.float32)



# revision 40
# speedup vs baseline: 1.2947x; 1.2947x over previous
"""Trainium2 Bass kernel for CompiledNCA — row-Toeplitz formulation.

Data parallel over batch (128 imgs -> 8 cores x 16; per core 2 groups of 8).
SBUF layout: partitions = (slot j, ci), j in [0,8); slot j of segment s holds
padded image row 6s + PI[j] with PI = [0,2,1,3,6,4,7,5]; free = (img, seg, x)
with x the 132-strided padded row (cols 0..129). Segments overlap 2 rows
(rows 6s, 6s+1 are stored twice); PI puts the 4 seam-row slots (rows 0,1,6,7)
at even slots so seam-replication copies start at 32-aligned partitions.

Each conv step + seg = 3 PSUM-accumulated full-array matmuls (one per
kernel-x tap dx, free-dim shifted); lhsT is a banded block-Toeplitz
[128=(jr,ci), 128=(jo,co)] holding W[co,ci,ky,dx], ky = PI[jr]-PI[jo]+1;
output slots that are not computable from the 8-row window (PI[jo] in {0,7})
have zero columns. One 512-col stream computes 6 output rows x 16 ch (2.18x
fewer PE cycles than a block-diagonal-per-image formulation). ReLU evac is a
full-partition psum->SBUF copy (no partition offset); seam rows are then
replicated by two batched fp16 SBUF->SBUF copies per image (which also
overwrite the zero columns with the true rows). Segment 21 (rows 127..128)
runs as its own N=128 matmul with a column-masked lhsT so halo slots stay 0.
Stem: dx-im2col'd x on partitions (yi,dx), 24-partition contract. Pool/fc:
ACT accum_out (DVE tensor_reduce for the DVE-evac'd groups; DVE accum_out is
low-precision) -> tensor_reduce -> one matmul + bias.

Schedule (PE was 91%-busy and DVE 88% in the v1 timeline sim; now PE ~98%):
- Evacs split ACT (seg-groups k<4) / DVE (k>=4 + both seam copies) so DVE is
  no longer the co-bottleneck. gpsimd cannot read PSUM, so it cannot help.
- Last step's h write is dead (only pooling reads it) and is recycled into
  the just-read src regions, freeing the other hbuf so group 1's stem
  (evac-bound: 1 matmul per seg-group vs 3) interleaves across group 0's
  t=14..15; group 0's stem staggers 2 images ahead of its own t=0.
- fc for group 0 is emitted during group 1's t=0; group 1's fc is split
  (imgs 0-5 early, 6-7 last) so only 2 images gate the final chain.
- In multi-rep NEFFs (timing), the next rep's group-0 stem interleaves into
  this rep's group-1 t=14..15, pipelining the rep boundary the same way.
- Startup: input DMA in 2-image slices ordered stem-first; hbuf memset
  narrowed to the pad columns; both group accumulators pre-allocated.
"""

import numpy as np

B, HC, OC, T = 128, 16, 10, 16
H = W = 128
NCORES = 8
IMGS = 8
GROUPS = 2
NSEG = 21                     # windows; base row = 6w+1 (head/tail emit 7)
XST = 132                     # x stride per (img,seg); cols 0..129 meaningful
SEGF = NSEG * XST             # 2904 free cols per img
MCOL = 128
PI = [0, 2, 1, 3, 6, 4, 7, 5]            # slot j -> row offset within segment
INV = [PI.index(r) for r in range(8)]    # row offset -> slot
SEG_T = [(0, 4), (4, 4), (8, 4), (12, 4), (16, 4), (20, 1)]
# t=T-1 needs masked interiors (pooling counts each row once): head, 5x
# interior, tail
SEG_T15 = [(0, 1), (1, 4), (5, 4), (9, 4), (13, 4), (17, 3), (20, 1)]

_cache = {}


def _view(hb, p0, np_, base, ns, x0, xn):
    """[np_, ns, xn] view of flat (seg,x) storage starting at free `base`."""
    v = hb[p0:p0 + np_, base:base + ns * XST]
    return v.rearrange("p (s x) -> p s x", x=XST)[:, :, x0:x0 + xn]


def _build(repeat=1):
    if repeat in _cache:
        return _cache[repeat]

    import concourse.bacc as bacc
    import concourse.mybir as mybir
    import concourse.tile as tile

    f16, f32 = mybir.dt.float16, mybir.dt.float32
    Relu = mybir.ActivationFunctionType.Relu
    Ident = mybir.ActivationFunctionType.Identity

    nc = bacc.Bacc("TRN2", target_bir_lowering=False, debug=False,
                   enable_asserts=False, num_devices=NCORES)

    d_w = nc.dram_tensor("w_steps", [128, ((T - 1) * 6 + 9) * 128], f16,
                     kind="ExternalInput")
    d_stemw = nc.dram_tensor("stem_w", [24, 2 * 128], f16, kind="ExternalInput")
    d_stemb = nc.dram_tensor("stem_b", [128, 2], f32, kind="ExternalInput")
    d_fcw = nc.dram_tensor("fc_w", [128, 128], f32, kind="ExternalInput")
    d_fcb = nc.dram_tensor("fc_b", [OC, 1], f32, kind="ExternalInput")
    d_xe = [nc.dram_tensor(f"x_exp{g}", [24, IMGS * SEGF], f16,
                           kind="ExternalInput") for g in range(GROUPS)]
    d_out = nc.dram_tensor("out", [GROUPS * OC, IMGS], f32, kind="ExternalOutput")

    with tile.TileContext(nc) as tc:
        with tc.tile_pool(name="const", bufs=1) as cp, \
             tc.tile_pool(name="hbuf", bufs=1) as hp, \
             tc.tile_pool(name="small", bufs=2) as sp, \
             tc.tile_pool(name="psum", bufs=8, space="PSUM") as pp:

            stemw_sb = cp.tile([24, 2 * 128], f16, tag="sw")
            nc.sync.dma_start(stemw_sb[:], d_stemw[:])
            stemb_sb = cp.tile([128, 2], f32, tag="sb")
            nc.sync.dma_start(stemb_sb[:], d_stemb[:])
            # per-image DMA slices so the stem can start after the first one
            xe_sb = []
            for g in range(GROUPS):
                t_ = cp.tile([24, IMGS * SEGF], f16, tag=f"xe{g}", name=f"xe{g}")
                xe_sb.append(t_)
            for i in range(0, IMGS, 2):
                nc.sync.dma_start(xe_sb[0][:, i * SEGF:(i + 2) * SEGF],
                                  d_xe[0][:, i * SEGF:(i + 2) * SEGF])
            w_tiles = []
            for t in range(T):
                nb = 9 if t == T - 1 else 6
                t_ = cp.tile([128, nb * 128], f16, tag=f"w{t}", name=f"w{t}")
                nc.sync.dma_start(
                    t_[:], d_w[:, t * 6 * 128:t * 6 * 128 + nb * 128])
                w_tiles.append(t_)
            for i in range(0, IMGS, 2):
                nc.sync.dma_start(xe_sb[1][:, i * SEGF:(i + 2) * SEGF],
                                  d_xe[1][:, i * SEGF:(i + 2) * SEGF])
            fcw_sb = cp.tile([128, 128], f32, tag="fw")
            nc.sync.dma_start(fcw_sb[:], d_fcw[:])
            fcb_sb = cp.tile([OC, 1], f32, tag="fb")
            nc.sync.dma_start(fcb_sb[:], d_fcb[:])

            hbufs = [hp.tile([128, IMGS * SEGF], f16, tag=f"h{i}", name=f"h{i}")
                     for i in range(2)]
            for hb in hbufs:
                # only pad cols 0 and 129 of each (img, seg) row need zeros;
                # everything else is written before it is read
                pads = hb[:].rearrange("p (q x) -> p q x", x=XST)
                nc.vector.memset(pads[:, :, 0:130:129], 0.0)

            accs = [sp.tile([128, IMGS * 7], f32, tag="acc", name=f"acc{g}")
                    for g in range(GROUPS)]


            def step_img(g, t, i):
                src = xe_sb[g] if t < 0 else hbufs[t % 2]
                # at t==T-1 the h write is dead (only pooling reads it), so
                # recycle the just-read src regions: keeps hbuf[(t+1)%2] free
                # for the next group's stem to overlap t=14..15
                dst = hbufs[(t + 1) % 2] if t != T - 1 else hbufs[t % 2]
                acc = accs[g]
                segs = SEG_T15 if t == T - 1 else SEG_T
                if True:
                    for k, (s0, ns) in enumerate(segs):
                        N = ns * 128
                        # variant: head-style (0) for all but the tail window
                        # group; at t==T-1: head(0)/masked-interior(1)/tail(2)
                        if t == T - 1:
                            sl = 0 if k == 0 else (2 if s0 == 20 else 1)
                        else:
                            sl = 1 if s0 == 20 else 0
                        ps = pp.tile([128, 512], f32, tag="ps")
                        if t < 0:
                            rhs = _view(src, 0, 24,
                                        i * SEGF + s0 * XST, ns, 0, 128)
                            nc.tensor.matmul(
                                ps[:, 0:N],
                                stemw_sb[:, sl * 128:sl * 128 + 128], rhs,
                                start=True, stop=True)
                        else:
                            for dx in range(3):
                                rhs = _view(src, 0, 128,
                                            i * SEGF + s0 * XST, ns, dx, 128)
                                wv = w_tiles[t][:, (sl * 3 + dx) * 128:
                                                (sl * 3 + dx + 1) * 128]
                                nc.tensor.matmul(ps[:, 0:N], wv, rhs,
                                                 start=(dx == 0),
                                                 stop=(dx == 2))
                        ps3 = ps[:, 0:N].rearrange("p (s x) -> p s x", x=128)
                        dmain = _view(dst, 0, 128,
                                      i * SEGF + s0 * XST, ns, 1, 128)
                        kwargs = {}
                        if t == T - 1:
                            kwargs = dict(
                                accum_out=acc[:, i * 7 + k:i * 7 + k + 1])
                        # evac split: ACT takes the first 4 seg-groups, DVE
                        # the rest (DVE also owns the seam copies); gpsimd
                        # cannot read PSUM so only ACT/DVE evacuate
                        if t < 0:
                            if k < 4:
                                nc.scalar.activation(
                                    dmain, ps3, Relu,
                                    bias=stemb_sb[:, sl:sl + 1], **kwargs)
                            else:
                                nc.vector.tensor_scalar(
                                    dmain, ps3, stemb_sb[:, sl:sl + 1], 0.0,
                                    mybir.AluOpType.add, mybir.AluOpType.max,
                                    **kwargs)
                        elif k < 4:
                            nc.scalar.activation(dmain, ps3, Relu, **kwargs)
                        elif t == T - 1:
                            # relu to the (dead) h slot, then DVE's
                            # fp32-internal tensor_reduce (accum_out on DVE
                            # is low-precision)
                            nc.vector.tensor_scalar_max(dmain, ps3, 0.0)
                            nc.vector.tensor_reduce(
                                acc[:, i * 7 + k:i * 7 + k + 1], dmain,
                                axis=mybir.AxisListType.XY,
                                op=mybir.AluOpType.add)
                        else:
                            nc.vector.tensor_scalar_max(dmain, ps3, 0.0)
                    if t == T - 1:
                        return  # h no longer read; pooling via accum_out
                    # seam replication (fp16 sbuf->sbuf, batched over segs);
                    # cols 0 and 129 are zero on both sides (memset pads), so
                    # copy only 1..128
                    # dup5: row 6s+6 (slot 4, s=0..20) -> slot 0 of seg s+1
                    nc.vector.tensor_copy(
                        _view(dst, 0, 16, i * SEGF + XST, 20, 1, 128),
                        _view(dst, 64, 16, i * SEGF, 20, 1, 128))
                    # dup0: row 6s+1 (slot 2, s=1..21) -> slot 6 of seg s-1
                    nc.vector.tensor_copy(
                        _view(dst, 96, 16, i * SEGF, 20, 1, 128),
                        _view(dst, 32, 16, i * SEGF + XST, 20, 1, 128))

            def fc_tail(g, lo=0, hi=IMGS, pooled=None, out_sb=None):
                n = hi - lo
                if pooled is None:
                    pooled = sp.tile([128, IMGS], f32, tag="pooled",
                                     name=f"pooled{g}")
                    out_sb = sp.tile([OC, IMGS], f32, tag="osb",
                                     name=f"osb{g}")
                nc.vector.tensor_reduce(
                    pooled[:, lo:hi],
                    accs[g][:, lo * 7:hi * 7].rearrange(
                        "p (i k) -> p i k", k=7),
                    axis=mybir.AxisListType.X, op=mybir.AluOpType.add)
                psfc = pp.tile([128, 512], f32, tag="ps", name=f"psfc{g}_{lo}")
                nc.tensor.matmul(psfc[:, 0:n], fcw_sb[:], pooled[:, lo:hi],
                                 start=True, stop=True)
                nc.scalar.activation(out_sb[:, lo:hi], psfc[0:OC, 0:n], Ident,
                                     bias=fcb_sb[:])
                nc.sync.dma_start(d_out[g * OC:(g + 1) * OC, lo:hi],
                                  out_sb[:, lo:hi])
                return pooled, out_sb

            for _rep in range(repeat):
                for g in range(GROUPS):
                    nc.vector.memset(accs[g][:], 0.0)
                if _rep == 0:
                    # first rep only: stem is evac-bound (1 matmul per group
                    # vs 3), so stagger t=0 two images behind the stem
                    step_img(0, -1, 0)
                    step_img(0, -1, 1)
                    for i in range(IMGS):
                        if i + 2 < IMGS:
                            step_img(0, -1, i + 2)
                        step_img(0, 0, i)
                else:
                    # stem already ran, interleaved into the previous rep's
                    # group-1 t=14..15
                    for i in range(IMGS):
                        step_img(0, 0, i)
                for t in range(1, T - 2):
                    for i in range(IMGS):
                        step_img(0, t, i)
                # group 1's stem (evac-bound) spreads over group 0's last two
                # steps; legal because t=15 writes recycle the src buffer
                for i in range(IMGS):
                    step_img(0, T - 2, i)
                    if i % 2 == 1:
                        step_img(1, -1, i // 2)
                for i in range(IMGS):
                    step_img(0, T - 1, i)
                    if i % 2 == 1:
                        step_img(1, -1, 4 + i // 2)
                for t in range(0, T - 2):
                    for i in range(IMGS):
                        step_img(1, t, i)
                    if t == 0:
                        fc_tail(0)  # overlaps group-1 compute
                # group-1 t=14..15 host the NEXT rep's group-0 stem (same
                # src-recycling argument as the group boundary above)
                nxt = _rep + 1 < repeat
                for i in range(IMGS):
                    step_img(1, T - 2, i)
                    if nxt and i % 2 == 1:
                        step_img(0, -1, i // 2)
                for i in range(IMGS):
                    step_img(1, T - 1, i)
                    if nxt and i % 2 == 1:
                        step_img(0, -1, 4 + i // 2)
                # fc for images 0..5 goes first (their accums are done while
                # imgs 6-7 compute); only images 6..7 gate the final chain
                p1, o1 = fc_tail(1, 0, 6)
                fc_tail(1, 6, IMGS, p1, o1)

    nc.compile()
    _cache[repeat] = nc
    return nc


def _prep_shared(stem_weight, stem_bias, weight_schedule, fc_weight, fc_bias):
    w = weight_schedule.astype(np.float32)          # [T, co, ci, 3, 3]
    # variants: 0 = head-style (outputs PI 0..6; PI=0 column uses ky 1,2
    # only, which the band formula yields automatically), 1 = masked
    # interior (1..6, t=T-1 only), 2 = tail (1..7; PI=7 uses ky 0,1)
    ROWS_OK = [(0, 1, 2, 3, 4, 5, 6), (1, 2, 3, 4, 5, 6), (1, 2, 3, 4, 5, 6, 7)]
    lhs = np.zeros((T, 3, 3, 128, 128), np.float16)
    for v in range(3):
        rows_ok = ROWS_OK[v]
        for dx in range(3):
            for jo in range(8):
                if PI[jo] not in rows_ok:
                    continue
                for jr in range(8):
                    ky = PI[jr] - PI[jo] + 1
                    if 0 <= ky <= 2:
                        lhs[:, v, dx, jr * HC:(jr + 1) * HC,
                            jo * HC:(jo + 1) * HC] = \
                            np.transpose(w[:, :, :, ky, dx], (0, 2, 1))
    # pack: steps 0..T-2 store [head, tail] (6 blocks); t=T-1 stores
    # [head, interior, tail] (9 blocks)
    cols = []
    for t in range(T):
        vs = (0, 1, 2) if t == T - 1 else (0, 2)
        for v in vs:
            for dx in range(3):
                cols.append(lhs[t, v, dx].T.T)  # [(jr,ci), (jo,co)]
    w_steps = np.ascontiguousarray(
        np.concatenate(cols, axis=1)).astype(np.float16)

    sw = stem_weight.astype(np.float32)             # [HC, 1, 3, 3]
    stem_lhs = np.zeros((24, 2, 128), np.float16)
    stem_b = np.zeros((128, 2), np.float32)
    for sl in range(2):
        rows_ok = (1, 2, 3, 4, 5, 6, 7) if sl else (0, 1, 2, 3, 4, 5, 6)
        for jo in range(8):
            if PI[jo] not in rows_ok:
                continue
            stem_b[jo * HC:(jo + 1) * HC, sl] = stem_bias
            for yi in range(8):
                ky = yi - PI[jo] + 1
                if 0 <= ky <= 2:
                    for dx in range(3):
                        stem_lhs[yi * 3 + dx, sl, jo * HC:(jo + 1) * HC] = \
                            sw[:, 0, ky, dx]
    stem_lhs = np.ascontiguousarray(stem_lhs.reshape(24, 256))

    fcw = np.tile(fc_weight.astype(np.float32).T / float(H * W), (8, 1))
    fcw = np.concatenate([fcw, np.zeros((128, 128 - OC), np.float32)], axis=1)
    fc_b = fc_bias.astype(np.float32)[:, None].copy()
    return {"w_steps": w_steps, "stem_w": stem_lhs, "stem_b": stem_b,
            "fc_w": fcw, "fc_b": fc_b}


def _prep_xexp(x_imgs):
    """Stem input: [yi*3+dx, (img, seg, 132)] fp16; AP x-offset j reads
    xpad[img, 6s+yi, j+dx]."""
    xpad = np.zeros((IMGS, H + 2, XST), np.float32)
    xpad[:, 1:1 + H, 1:1 + W] = x_imgs[:, 0]
    out = np.zeros((24, IMGS * SEGF), np.float16)
    for i in range(IMGS):
        for yi in range(8):
            for dx in range(3):
                p = yi * 3 + dx
                for s in range(NSEG):
                    r = 6 * s + yi + 1
                    if r >= H + 2:
                        continue
                    col = i * SEGF + s * XST
                    out[p, col:col + 130 - dx] = xpad[i, r, dx:130]
    return out


def kernel(x, stem_weight, stem_bias, weight_schedule, fc_weight, fc_bias):
    from concourse.bass_utils import run_bass_kernel_spmd

    x = np.asarray(x, dtype=np.float32)
    nc = _build()
    shared = _prep_shared(np.asarray(stem_weight, np.float32),
                          np.asarray(stem_bias, np.float32),
                          np.asarray(weight_schedule, np.float32),
                          np.asarray(fc_weight, np.float32),
                          np.asarray(fc_bias, np.float32))
    in_maps = []
    for c in range(NCORES):
        m = dict(shared)
        for g in range(GROUPS):
            lo = c * GROUPS * IMGS + g * IMGS
            m[f"x_exp{g}"] = _prep_xexp(x[lo:lo + IMGS])
        in_maps.append(m)

    res = run_bass_kernel_spmd(nc, in_maps, core_ids=list(range(NCORES)),
                               trace=False)
    outs = []
    for c in range(NCORES):
        o = res.results[c]["out"]          # [G*OC, IMGS]
        for g in range(GROUPS):
            outs.append(o[g * OC:(g + 1) * OC, :].T)
    return np.concatenate(outs, axis=0).astype(np.float32)



# revision 43
# speedup vs baseline: 1.3113x; 1.0128x over previous
"""Trainium2 Bass kernel for CompiledNCA — row-Toeplitz formulation.

Data parallel over batch (128 imgs -> 8 cores x 16; per core 2 groups of 8).
SBUF layout: partitions = (slot j, ci), j in [0,8); slot j of segment s holds
padded image row 6s + PI[j] with PI = [0,2,1,3,6,4,7,5]; free = (img, seg, x)
with x the 132-strided padded row (cols 0..129). Segments overlap 2 rows
(rows 6s, 6s+1 are stored twice); PI puts the 4 seam-row slots (rows 0,1,6,7)
at even slots so seam-replication copies start at 32-aligned partitions.

Each conv step + seg = 3 PSUM-accumulated full-array matmuls (one per
kernel-x tap dx, free-dim shifted); lhsT is a banded block-Toeplitz
[128=(jr,ci), 128=(jo,co)] holding W[co,ci,ky,dx], ky = PI[jr]-PI[jo]+1;
output slots that are not computable from the 8-row window (PI[jo] in {0,7})
have zero columns. One 512-col stream computes 6 output rows x 16 ch (2.18x
fewer PE cycles than a block-diagonal-per-image formulation). ReLU evac is a
full-partition psum->SBUF copy (no partition offset); seam rows are then
replicated by two batched fp16 SBUF->SBUF copies per image (which also
overwrite the zero columns with the true rows). Segment 21 (rows 127..128)
runs as its own N=128 matmul. Windows are based at padded row 6w+1 (21
windows, not 22): the head window emits 7 rows (its first output's ky=0 tap
is the zero pad row, absent from the band automatically) and the tail emits
7 (last output's ky=2 tap is the bottom pad); interior windows' PI=0/PI=7
psum columns are garbage partials overwritten by the dups before any read.
Only t=T-1 (pooling) uses a 3-variant split so each row is counted once.
Stem: dx-im2col'd x on partitions (yi,dx), 24-partition contract. Pool/fc:
ACT accum_out (DVE tensor_reduce for the DVE-evac'd groups; DVE accum_out is
low-precision) -> tensor_reduce -> one matmul + bias.

Schedule (PE was 91%-busy and DVE 88% in the v1 timeline sim; now PE ~98%):
- Evacs split ACT (seg-groups k<4) / DVE (k>=4 + both seam copies) so DVE is
  no longer the co-bottleneck. gpsimd cannot read PSUM, so it cannot help.
- Last step's h write is dead (only pooling reads it) and is recycled into
  the just-read src regions, freeing the other hbuf so group 1's stem
  (evac-bound: 1 matmul per seg-group vs 3) interleaves across group 0's
  t=14..15; group 0's stem staggers 2 images ahead of its own t=0.
- fc for group 0 is emitted during group 1's t=0; group 1's fc is split
  (imgs 0-5 early, 6-7 last) so only 2 images gate the final chain.
- In multi-rep NEFFs (timing), the next rep's group-0 stem interleaves into
  this rep's group-1 t=14..15, pipelining the rep boundary the same way.
- Startup: input DMA in 2-image slices ordered stem-first; hbuf memset
  narrowed to the pad columns; both group accumulators pre-allocated.
"""

import numpy as np

B, HC, OC, T = 128, 16, 10, 16
H = W = 128
NCORES = 8
IMGS = 8
GROUPS = 2
NSEG = 21                     # windows; base row = 6w+1 (head/tail emit 7)
XST = 132                     # x stride per (img,seg); cols 0..129 meaningful
SEGF = NSEG * XST             # 2904 free cols per img
MCOL = 128
PI = [0, 2, 1, 3, 6, 4, 7, 5]            # slot j -> row offset within segment
INV = [PI.index(r) for r in range(8)]    # row offset -> slot
SEG_T = [(0, 4), (4, 4), (8, 4), (12, 4), (16, 4), (20, 1)]
# t=T-1 needs masked interiors (pooling counts each row once): head, 5x
# interior, tail
SEG_T15 = [(0, 1), (1, 4), (5, 4), (9, 4), (13, 4), (17, 3), (20, 1)]

_cache = {}


def _view(hb, p0, np_, base, ns, x0, xn):
    """[np_, ns, xn] view of flat (seg,x) storage starting at free `base`."""
    v = hb[p0:p0 + np_, base:base + ns * XST]
    return v.rearrange("p (s x) -> p s x", x=XST)[:, :, x0:x0 + xn]


def _build(repeat=1):
    if repeat in _cache:
        return _cache[repeat]

    import concourse.bacc as bacc
    import concourse.mybir as mybir
    import concourse.tile as tile

    f16, f32 = mybir.dt.float16, mybir.dt.float32
    Relu = mybir.ActivationFunctionType.Relu
    Ident = mybir.ActivationFunctionType.Identity

    nc = bacc.Bacc("TRN2", target_bir_lowering=False, debug=False,
                   enable_asserts=False, num_devices=NCORES)

    d_w = nc.dram_tensor("w_steps", [128, ((T - 1) * 6 + 9) * 128], f16,
                     kind="ExternalInput")
    d_stemw = nc.dram_tensor("stem_w", [24, 2 * 128], f16, kind="ExternalInput")
    d_stemb = nc.dram_tensor("stem_b", [128, 2], f32, kind="ExternalInput")
    d_fcw = nc.dram_tensor("fc_w", [128, 128], f32, kind="ExternalInput")
    d_fcb = nc.dram_tensor("fc_b", [OC, 1], f32, kind="ExternalInput")
    d_xe = [nc.dram_tensor(f"x_exp{g}", [24, IMGS * SEGF], f16,
                           kind="ExternalInput") for g in range(GROUPS)]
    d_out = nc.dram_tensor("out", [GROUPS * OC, IMGS], f32, kind="ExternalOutput")

    with tile.TileContext(nc) as tc:
        with tc.tile_pool(name="const", bufs=1) as cp, \
             tc.tile_pool(name="hbuf", bufs=1) as hp, \
             tc.tile_pool(name="small", bufs=2) as sp, \
             tc.tile_pool(name="psum", bufs=8, space="PSUM") as pp:

            stemw_sb = cp.tile([24, 2 * 128], f16, tag="sw")
            nc.sync.dma_start(stemw_sb[:], d_stemw[:])
            stemb_sb = cp.tile([128, 2], f32, tag="sb")
            nc.sync.dma_start(stemb_sb[:], d_stemb[:])
            # per-image DMA slices so the stem can start after the first one
            xe_sb = []
            for g in range(GROUPS):
                t_ = cp.tile([24, IMGS * SEGF], f16, tag=f"xe{g}", name=f"xe{g}")
                xe_sb.append(t_)
            for i in range(0, IMGS, 2):
                nc.sync.dma_start(xe_sb[0][:, i * SEGF:(i + 2) * SEGF],
                                  d_xe[0][:, i * SEGF:(i + 2) * SEGF])
            w_tiles = []
            for t in range(T):
                nb = 9 if t == T - 1 else 6
                t_ = cp.tile([128, nb * 128], f16, tag=f"w{t}", name=f"w{t}")
                nc.sync.dma_start(
                    t_[:], d_w[:, t * 6 * 128:t * 6 * 128 + nb * 128])
                w_tiles.append(t_)
            for i in range(0, IMGS, 2):
                nc.sync.dma_start(xe_sb[1][:, i * SEGF:(i + 2) * SEGF],
                                  d_xe[1][:, i * SEGF:(i + 2) * SEGF])
            fcw_sb = cp.tile([128, 128], f32, tag="fw")
            nc.sync.dma_start(fcw_sb[:], d_fcw[:])
            fcb_sb = cp.tile([OC, 1], f32, tag="fb")
            nc.sync.dma_start(fcb_sb[:], d_fcb[:])

            hbufs = [hp.tile([128, IMGS * SEGF], f16, tag=f"h{i}", name=f"h{i}")
                     for i in range(2)]
            for hb in hbufs:
                # only pad cols 0 and 129 of each (img, seg) row need zeros;
                # everything else is written before it is read
                pads = hb[:].rearrange("p (q x) -> p q x", x=XST)
                nc.vector.memset(pads[:, :, 0:130:129], 0.0)

            accs = [sp.tile([128, IMGS * 7], f32, tag="acc", name=f"acc{g}")
                    for g in range(GROUPS)]


            def step_img(g, t, i):
                src = xe_sb[g] if t < 0 else hbufs[t % 2]
                # at t==T-1 the h write is dead (only pooling reads it), so
                # recycle the just-read src regions: keeps hbuf[(t+1)%2] free
                # for the next group's stem to overlap t=14..15
                dst = hbufs[(t + 1) % 2] if t != T - 1 else hbufs[t % 2]
                acc = accs[g]
                segs = SEG_T15 if t == T - 1 else SEG_T
                if True:
                    for k, (s0, ns) in enumerate(segs):
                        N = ns * 128
                        # variant: head-style (0) for all but the tail window
                        # group; at t==T-1: head(0)/masked-interior(1)/tail(2)
                        if t == T - 1:
                            sl = 0 if k == 0 else (2 if s0 == 20 else 1)
                        else:
                            sl = 1 if s0 == 20 else 0
                        ps = pp.tile([128, 512], f32, tag="ps")
                        if t < 0:
                            rhs = _view(src, 0, 24,
                                        i * SEGF + s0 * XST, ns, 0, 128)
                            nc.tensor.matmul(
                                ps[:, 0:N],
                                stemw_sb[:, sl * 128:sl * 128 + 128], rhs,
                                start=True, stop=True)
                        else:
                            for dx in range(3):
                                rhs = _view(src, 0, 128,
                                            i * SEGF + s0 * XST, ns, dx, 128)
                                wv = w_tiles[t][:, (sl * 3 + dx) * 128:
                                                (sl * 3 + dx + 1) * 128]
                                nc.tensor.matmul(ps[:, 0:N], wv, rhs,
                                                 start=(dx == 0),
                                                 stop=(dx == 2))
                        ps3 = ps[:, 0:N].rearrange("p (s x) -> p s x", x=128)
                        dmain = _view(dst, 0, 128,
                                      i * SEGF + s0 * XST, ns, 1, 128)
                        kwargs = {}
                        if t == T - 1:
                            kwargs = dict(
                                accum_out=acc[:, i * 7 + k:i * 7 + k + 1])
                        # evac split: ACT takes the first 4 seg-groups, DVE
                        # the rest (DVE also owns the seam copies); gpsimd
                        # cannot read PSUM so only ACT/DVE evacuate
                        if t < 0:
                            if k < 4:
                                nc.scalar.activation(
                                    dmain, ps3, Relu,
                                    bias=stemb_sb[:, sl:sl + 1], **kwargs)
                            else:
                                nc.vector.tensor_scalar(
                                    dmain, ps3, stemb_sb[:, sl:sl + 1], 0.0,
                                    mybir.AluOpType.add, mybir.AluOpType.max,
                                    **kwargs)
                        elif k < 4:
                            nc.scalar.activation(dmain, ps3, Relu, **kwargs)
                        elif t == T - 1:
                            # relu to the (dead) h slot, then DVE's
                            # fp32-internal tensor_reduce (accum_out on DVE
                            # is low-precision)
                            nc.vector.tensor_scalar_max(dmain, ps3, 0.0)
                            nc.vector.tensor_reduce(
                                acc[:, i * 7 + k:i * 7 + k + 1], dmain,
                                axis=mybir.AxisListType.XY,
                                op=mybir.AluOpType.add)
                        else:
                            nc.vector.tensor_scalar_max(dmain, ps3, 0.0)
                    if t == T - 1:
                        return  # h no longer read; pooling via accum_out
                    # seam replication (fp16 sbuf->sbuf, batched over segs);
                    # cols 0 and 129 are zero on both sides (memset pads), so
                    # copy only 1..128
                    # dup5: row 6s+6 (slot 4, s=0..20) -> slot 0 of seg s+1
                    nc.vector.tensor_copy(
                        _view(dst, 0, 16, i * SEGF + XST, 20, 1, 128),
                        _view(dst, 64, 16, i * SEGF, 20, 1, 128))
                    # dup0: row 6s+1 (slot 2, s=1..21) -> slot 6 of seg s-1
                    nc.vector.tensor_copy(
                        _view(dst, 96, 16, i * SEGF, 20, 1, 128),
                        _view(dst, 32, 16, i * SEGF + XST, 20, 1, 128))

            def fc_tail(g, lo=0, hi=IMGS, pooled=None, out_sb=None):
                n = hi - lo
                if pooled is None:
                    pooled = sp.tile([128, IMGS], f32, tag="pooled",
                                     name=f"pooled{g}")
                    out_sb = sp.tile([OC, IMGS], f32, tag="osb",
                                     name=f"osb{g}")
                nc.vector.tensor_reduce(
                    pooled[:, lo:hi],
                    accs[g][:, lo * 7:hi * 7].rearrange(
                        "p (i k) -> p i k", k=7),
                    axis=mybir.AxisListType.X, op=mybir.AluOpType.add)
                psfc = pp.tile([128, 512], f32, tag="ps", name=f"psfc{g}_{lo}")
                nc.tensor.matmul(psfc[:, 0:n], fcw_sb[:], pooled[:, lo:hi],
                                 start=True, stop=True)
                nc.scalar.activation(out_sb[:, lo:hi], psfc[0:OC, 0:n], Ident,
                                     bias=fcb_sb[:])
                nc.sync.dma_start(d_out[g * OC:(g + 1) * OC, lo:hi],
                                  out_sb[:, lo:hi])
                return pooled, out_sb

            for _rep in range(repeat):
                for g in range(GROUPS):
                    nc.vector.memset(accs[g][:], 0.0)
                if _rep == 0:
                    # first rep only: stem is evac-bound (1 matmul per group
                    # vs 3), so stagger t=0 two images behind the stem
                    step_img(0, -1, 0)
                    step_img(0, -1, 1)
                    for i in range(IMGS):
                        if i + 2 < IMGS:
                            step_img(0, -1, i + 2)
                        step_img(0, 0, i)
                else:
                    # stem already ran, interleaved into the previous rep's
                    # group-1 t=14..15
                    for i in range(IMGS):
                        step_img(0, 0, i)
                for t in range(1, T - 2):
                    for i in range(IMGS):
                        step_img(0, t, i)
                # group 1's stem (evac-bound) spreads over group 0's last two
                # steps; legal because t=15 writes recycle the src buffer
                for i in range(IMGS):
                    step_img(0, T - 2, i)
                    if i % 2 == 1:
                        step_img(1, -1, i // 2)
                for i in range(IMGS):
                    step_img(0, T - 1, i)
                    if i % 2 == 1:
                        step_img(1, -1, 4 + i // 2)
                for t in range(0, T - 2):
                    for i in range(IMGS):
                        step_img(1, t, i)
                    if t == 0:
                        fc_tail(0)  # overlaps group-1 compute
                # group-1 t=14..15 host the NEXT rep's group-0 stem (same
                # src-recycling argument as the group boundary above)
                nxt = _rep + 1 < repeat
                for i in range(IMGS):
                    step_img(1, T - 2, i)
                    if nxt and i % 2 == 1:
                        step_img(0, -1, i // 2)
                for i in range(IMGS):
                    step_img(1, T - 1, i)
                    if nxt and i % 2 == 1:
                        step_img(0, -1, 4 + i // 2)
                # fc for images 0..5 goes first (their accums are done while
                # imgs 6-7 compute); only images 6..7 gate the final chain
                p1, o1 = fc_tail(1, 0, 7)
                fc_tail(1, 7, IMGS, p1, o1)

    nc.compile()
    _cache[repeat] = nc
    return nc


def _prep_shared(stem_weight, stem_bias, weight_schedule, fc_weight, fc_bias):
    w = weight_schedule.astype(np.float32)          # [T, co, ci, 3, 3]
    # variants: 0 = head-style (outputs PI 0..6; PI=0 column uses ky 1,2
    # only, which the band formula yields automatically), 1 = masked
    # interior (1..6, t=T-1 only), 2 = tail (1..7; PI=7 uses ky 0,1)
    ROWS_OK = [(0, 1, 2, 3, 4, 5, 6), (1, 2, 3, 4, 5, 6), (1, 2, 3, 4, 5, 6, 7)]
    lhs = np.zeros((T, 3, 3, 128, 128), np.float16)
    for v in range(3):
        rows_ok = ROWS_OK[v]
        for dx in range(3):
            for jo in range(8):
                if PI[jo] not in rows_ok:
                    continue
                for jr in range(8):
                    ky = PI[jr] - PI[jo] + 1
                    if 0 <= ky <= 2:
                        lhs[:, v, dx, jr * HC:(jr + 1) * HC,
                            jo * HC:(jo + 1) * HC] = \
                            np.transpose(w[:, :, :, ky, dx], (0, 2, 1))
    # pack: steps 0..T-2 store [head, tail] (6 blocks); t=T-1 stores
    # [head, interior, tail] (9 blocks)
    cols = []
    for t in range(T):
        vs = (0, 1, 2) if t == T - 1 else (0, 2)
        for v in vs:
            for dx in range(3):
                cols.append(lhs[t, v, dx].T.T)  # [(jr,ci), (jo,co)]
    w_steps = np.ascontiguousarray(
        np.concatenate(cols, axis=1)).astype(np.float16)

    sw = stem_weight.astype(np.float32)             # [HC, 1, 3, 3]
    stem_lhs = np.zeros((24, 2, 128), np.float16)
    stem_b = np.zeros((128, 2), np.float32)
    for sl in range(2):
        rows_ok = (1, 2, 3, 4, 5, 6, 7) if sl else (0, 1, 2, 3, 4, 5, 6)
        for jo in range(8):
            if PI[jo] not in rows_ok:
                continue
            stem_b[jo * HC:(jo + 1) * HC, sl] = stem_bias
            for yi in range(8):
                ky = yi - PI[jo] + 1
                if 0 <= ky <= 2:
                    for dx in range(3):
                        stem_lhs[yi * 3 + dx, sl, jo * HC:(jo + 1) * HC] = \
                            sw[:, 0, ky, dx]
    stem_lhs = np.ascontiguousarray(stem_lhs.reshape(24, 256))

    fcw = np.tile(fc_weight.astype(np.float32).T / float(H * W), (8, 1))
    fcw = np.concatenate([fcw, np.zeros((128, 128 - OC), np.float32)], axis=1)
    fc_b = fc_bias.astype(np.float32)[:, None].copy()
    return {"w_steps": w_steps, "stem_w": stem_lhs, "stem_b": stem_b,
            "fc_w": fcw, "fc_b": fc_b}


def _prep_xexp(x_imgs):
    """Stem input: [yi*3+dx, (img, seg, 132)] fp16; AP x-offset j reads
    xpad[img, 6s+yi, j+dx]."""
    xpad = np.zeros((IMGS, H + 2, XST), np.float32)
    xpad[:, 1:1 + H, 1:1 + W] = x_imgs[:, 0]
    out = np.zeros((24, IMGS * SEGF), np.float16)
    for i in range(IMGS):
        for yi in range(8):
            for dx in range(3):
                p = yi * 3 + dx
                for s in range(NSEG):
                    r = 6 * s + yi + 1
                    if r >= H + 2:
                        continue
                    col = i * SEGF + s * XST
                    out[p, col:col + 130 - dx] = xpad[i, r, dx:130]
    return out


def kernel(x, stem_weight, stem_bias, weight_schedule, fc_weight, fc_bias):
    from concourse.bass_utils import run_bass_kernel_spmd

    x = np.asarray(x, dtype=np.float32)
    nc = _build()
    shared = _prep_shared(np.asarray(stem_weight, np.float32),
                          np.asarray(stem_bias, np.float32),
                          np.asarray(weight_schedule, np.float32),
                          np.asarray(fc_weight, np.float32),
                          np.asarray(fc_bias, np.float32))
    in_maps = []
    for c in range(NCORES):
        m = dict(shared)
        for g in range(GROUPS):
            lo = c * GROUPS * IMGS + g * IMGS
            m[f"x_exp{g}"] = _prep_xexp(x[lo:lo + IMGS])
        in_maps.append(m)

    res = run_bass_kernel_spmd(nc, in_maps, core_ids=list(range(NCORES)),
                               trace=False)
    outs = []
    for c in range(NCORES):
        o = res.results[c]["out"]          # [G*OC, IMGS]
        for g in range(GROUPS):
            outs.append(o[g * OC:(g + 1) * OC, :].T)
    return np.concatenate(outs, axis=0).astype(np.float32)

